# revision 1
# baseline (speedup 1.0000x reference)
"""BiLSTM classifier on 8 TRN2 NeuronCores.

Sharding: batch 4-way x direction 2-way. Core r handles batch quarter
q = r//2 (16 sequences) and LSTM direction d = r%2 for BOTH layers.
Backward-direction cores see their input time-reversed so the device
program is identical on all cores (pure SPMD). Layer-0 hidden states are
exchanged between the (fwd, bwd) core pairs with per-chunk AllGathers so
each core can build the concat(fwd, bwd) input projection for layer 1.
Each core returns its 16 sequences' partial FC output (own direction's
half of the pooled features); host adds the pair and concatenates.

Step math: all four gates go through ONE sigmoid (g-gate rows of the
weights/biases are pre-scaled by 2 on the host; tanh(x) = 2*sig(2x)-1
is reconstructed on the vector engine). The per-step input projection
xp is accumulated into the gates PSUM with an identity matmul, and the
per-core batch is split into two groups with independent chains so the
elementwise latency of one group hides under the other's matmuls.
"""
import sys

if '/opt/trn_rl_repo' not in sys.path:
    sys.path.insert(0, '/opt/trn_rl_repo')

import numpy as np

import concourse.bass as bass
import concourse.mybir as mybir
from concourse import tile
from concourse.bass_utils import run_bass_kernel_spmd
from concourse.vector_clock import ScopedClock

B, T, DIN, H, NCLS = 64, 1024, 12, 256, 17
NCORES = 8
BSH = B // (NCORES // 2)      # 16 sequences per core
NG = 2                        # batch groups per core (independent chains)
GB = BSH // NG                # 8 sequences per group
CHUNK = 64
H4 = 4 * H                    # 1024
KH = H // 128                 # 2 k-tiles per hidden state
M4 = H4 // 128                # 8 m-tiles of gates
F32 = mybir.dt.float32
BF16 = mybir.dt.bfloat16
AF = mybir.ActivationFunctionType
ALU = mybir.AluOpType


def _patch_tile_drain():
    """Walrus in this env rejects >1 sync-wait on one instruction; spread the
    final Tile drain's waits across sync-engine nops."""
    def _drain_and_barrier(self, tick_clock, wait_clock):
        drain_inst = self.nc.sync.drain()
        wait_clock.add_sem_waits(
            drain_inst.ins, ScopedClock({None: tick_clock.global_clock}))
        si = drain_inst.ins.sync_info
        if si is not None and len(si.on_wait) > 1:
            waits = list(si.on_wait)
            drain_inst.ins.sync_info = mybir.SyncInfo(
                on_wait=[waits[0]], on_update=list(si.on_update))
            for w in waits[1:]:
                nop = self.nc.sync.nop(nofuse=True)
                nop.ins.sync_info = mybir.SyncInfo(on_wait=[w], on_update=[])
        self.nc.all_engine_barrier()
        assert self.sems is not None
        popped = self.nc._tile_sem_poison_stack.pop()
        assert popped is self._sem_poison
        self.nc.clear_and_free_semaphores(list(self.sems.allocated().values()))
        self.nc.all_engine_barrier()

    tile.TileContext._drain_and_barrier = _drain_and_barrier


_patch_tile_drain()


def _split_multi_waits(nc):
    """This env's walrus supports only one sync-wait per instruction: move
    extra waits onto same-engine nops inserted just before the instruction."""
    cnt = 0
    for fn in nc.m.functions:
        for bb in fn.blocks:
            new = []
            changed = False
            for inst in bb.instructions:
                si = inst.sync_info
                if si is not None and len(si.on_wait) > 1:
                    changed = True
                    waits = list(si.on_wait)
                    for w in waits[:-1]:
                        nop = mybir.InstNoOp(
                            name=f"waitsplit_{cnt}", ins=[], outs=[])
                        cnt += 1
                        nop.engine = inst.engine
                        nop.sync_info = mybir.SyncInfo(
                            on_wait=[w], on_update=[])
                        new.append(nop)
                    inst.sync_info = mybir.SyncInfo(
                        on_wait=[waits[-1]], on_update=list(si.on_update))
                new.append(inst)
            if changed:
                bb.instructions = new


def build_nc(t_len=T, use_collective=True):
    nch = t_len // CHUNK
    nc = bass.Bass(num_devices=NCORES)

    # ---- external parameters (per-core data, identical program) ----
    xT_ext = nc.declare_dram_parameter("xT", [DIN + 1, t_len * BSH], F32,
                                       isOutput=False)
    whh0_ext = nc.declare_dram_parameter("whh0T", [H, H4], F32, isOutput=False)
    wih0_ext = nc.declare_dram_parameter("wih0T", [DIN + 1, H4], F32,
                                         isOutput=False)
    whh1_ext = nc.declare_dram_parameter("whh1T", [H, H4], F32, isOutput=False)
    wih1_ext = nc.declare_dram_parameter("wih1Te", [3 * H, H4], F32,
                                         isOutput=False)
    b1_ext = nc.declare_dram_parameter("b1", [1, H4], F32, isOutput=False)
    ident_ext = nc.declare_dram_parameter("ident", [128, 128], F32,
                                          isOutput=False)
    fcw_ext = nc.declare_dram_parameter("fcwT", [H, NCLS], F32, isOutput=False)
    fcb_ext = nc.declare_dram_parameter("fcb", [BSH, NCLS], F32, isOutput=False)
    out_ext = nc.declare_dram_parameter("out", [BSH, NCLS], F32, isOutput=True)

    # ---- dram scratch for the pairwise h0 exchange ----
    excin = []
    excout = []
    for c in range(nch):
        excin.append(nc.dram_tensor(f"excin{c}", [128, KH, CHUNK, BSH], BF16))
        excout.append(
            nc.dram_tensor(f"excout{c}", [2, 128, KH, CHUNK, BSH], BF16))
    groups = [[2 * q, 2 * q + 1] for q in range(NCORES // 2)]

    with tile.TileContext(nc) as tc:
        with (
            tc.tile_pool(name="const", bufs=1) as constp,
            tc.tile_pool(name="stage", bufs=2) as stagep,
            tc.tile_pool(name="xp", bufs=2) as xpp,
            tc.tile_pool(name="hg", bufs=4) as hgp,
            tc.tile_pool(name="step", bufs=4) as stepp,
            tc.tile_pool(name="gp", bufs=2, space=bass.MemorySpace.PSUM) as gpp,
            tc.tile_pool(name="xps", bufs=2, space=bass.MemorySpace.PSUM) as xpsp,
        ):
            # ---------- load + cast weights ----------
            def load_bf16(ext, rows, cols, tag):
                if rows <= 128:
                    out_t = constp.tile([rows, cols], BF16, tag=tag)
                    st = stagep.tile([128, cols], F32, tag="wstage")
                    nc.sync.dma_start(st[:rows, :], ext[:, :])
                    nc.vector.tensor_copy(out_t[:], st[:rows, :])
                else:
                    out_t = constp.tile([128, rows // 128, cols], BF16, tag=tag)
                    for i in range(rows // 128):
                        st = stagep.tile([128, cols], F32, tag="wstage")
                        nc.sync.dma_start(st[:], ext[i * 128:(i + 1) * 128, :])
                        nc.vector.tensor_copy(out_t[:, i, :], st[:])
                return out_t

            whh0_sb = load_bf16(whh0_ext, H, H4, "whh0")   # [128, KH, H4]
            whh1_sb = load_bf16(whh1_ext, H, H4, "whh1")
            wih1_sb = load_bf16(wih1_ext, 3 * H, H4, "wih1")  # [128, 6, H4]
            wih0_sb = load_bf16(wih0_ext, DIN + 1, H4, "wih0")  # [13, H4]
            b1_sb = load_bf16(b1_ext, 1, H4, "b1")         # [1, H4]
            fcw_sb = load_bf16(fcw_ext, H, NCLS, "fcw")    # [128, KH, NCLS]

            ident_sb = load_bf16(ident_ext, 128, 128, "ident")
            fcb_sb = constp.tile([BSH, NCLS], F32)
            nc.sync.dma_start(fcb_sb[:], fcb_ext[:])
            ones_sb = constp.tile([1, 512], BF16)
            nc.gpsimd.memset(ones_sb[:], 1.0)

            xT_sb = constp.tile([DIN + 1, t_len * BSH], BF16)
            for j in range(t_len * BSH // 1024):
                st = stagep.tile([128, 1024], F32, tag="wstage")
                nc.sync.dma_start(st[:DIN + 1, :],
                                  xT_ext[:, j * 1024:(j + 1) * 1024])
                nc.vector.tensor_copy(xT_sb[:, j * 1024:(j + 1) * 1024],
                                      st[:DIN + 1, :])

            # ---------- persistent state ----------
            # h0 store: slot s = 1 + local step; slot 0 is the zero init.
            h0_sb = constp.tile([128, KH, t_len + 1, BSH], BF16)
            nc.gpsimd.memset(h0_sb[:, :, 0, :], 0.0)
            h1r = constp.tile([128, 2, KH, BSH], BF16)   # layer-1 h ring
            nc.gpsimd.memset(h1r[:, 1, :, :], 0.0)
            c_state = constp.tile([128, KH, BSH], F32)
            h1sum = constp.tile([128, KH, BSH], F32)
            nc.gpsimd.memset(h1sum[:], 0.0)

            def xp_gemm(layer, c):
                """Input-projection chunk (bias folded in): returns
                xp tile [128, M4, CHUNK, BSH] f32 (drained from PSUM by DMA)."""
                xp_t = xpp.tile([128, M4, CHUNK, BSH], BF16, tag="xp")
                if layer == 1:
                    hg0 = hgp.tile([128, KH, CHUNK, BSH], BF16, tag="hg")
                    hg1 = hgp.tile([128, KH, CHUNK, BSH], BF16, tag="hg")
                    src = excout[nch - 1 - c]
                    nc.sync.dma_start(hg0[:], src[0][:, :, ::-1, :])
                    nc.sync.dma_start(hg1[:], src[1][:, :, ::-1, :])
                ncols = CHUNK * BSH                   # 1024
                tn = 512 // BSH                       # timesteps per matmul
                for m in range(M4):
                    for half in range(ncols // 512):
                        t0 = half * tn
                        ps = xpsp.tile([128, 512], F32, tag="xps")
                        if layer == 0:
                            nc.tensor.matmul(
                                ps[:],
                                wih0_sb[:, m * 128:(m + 1) * 128],
                                xT_sb[:, c * ncols + half * 512:
                                      c * ncols + half * 512 + 512],
                                start=True, stop=True)
                        else:
                            rhss = [h0_sb[:, k, 1 + c * CHUNK + t0:
                                          1 + c * CHUNK + t0 + tn, :]
                                    for k in range(KH)]
                            for hg in (hg0, hg1):
                                for k in range(KH):
                                    rhss.append(hg[:, k, t0:t0 + tn, :])
                            for kk in range(6):
                                nc.tensor.matmul(
                                    ps[:],
                                    wih1_sb[:, kk, m * 128:(m + 1) * 128],
                                    rhss[kk],
                                    start=(kk == 0), stop=False)
                            # bias via K=1 matmul against a ones row
                            nc.tensor.matmul(
                                ps[:], b1_sb[:, m * 128:(m + 1) * 128],
                                ones_sb[:1, :],
                                start=False, stop=True)
                        # drain PSUM -> SBUF; alternate engines for balance
                        dst = xp_t[:, m, t0:t0 + tn, :].rearrange(
                            "p c g -> p (c g)")
                        if m % 2 == 0:
                            nc.scalar.activation(dst, ps[:], AF.Copy)
                        else:
                            nc.vector.tensor_copy(dst, ps[:])
                return xp_t

            def lstm_step(layer, tau, xp_t, whh_sb):
                """One timestep for all NG groups, software-pipelined: group
                A's full burst is emitted before group B's so A's chain can
                weave under B's matmuls; the two chains are interleaved
                op-by-op so neither blocks the other in the engine queues."""
                tl = tau % CHUNK
                gps, gacts = [], []
                for g in range(NG):
                    gp = gpp.tile([128, M4, GB], F32, tag=f"gp{g}",
                                  name=f"gp{g}")
                    # seed the gates psum with xp via one identity MM, then
                    # let the recurrent matmuls accumulate on top
                    nc.tensor.matmul(
                        gp[:], ident_sb[:],
                        xp_t[:, :, tl, g * GB:(g + 1) * GB],
                        start=True, stop=False)
                    for m in range(M4):
                        for k in range(KH):
                            if layer == 0:
                                rhs = h0_sb[:, k, tau, g * GB:(g + 1) * GB]
                            else:
                                rhs = h1r[:, (tau + 1) % 2, k,
                                          g * GB:(g + 1) * GB]
                            nc.tensor.matmul(
                                gp[:, m, :],
                                whh_sb[:, k, m * 128:(m + 1) * 128],
                                rhs,
                                start=False,
                                stop=(m == M4 - 1 and k == KH - 1))
                    gps.append(gp)
                    gact = stepp.tile([128, M4, GB], BF16, tag=f"gact{g}",
                                      name=f"gact{g}")
                    nc.scalar.activation(gact[:], gp[:], AF.Sigmoid)
                    gacts.append(gact)
                tgs, igs, fcs, tcs = [], [], [], []
                for g in range(NG):
                    # tanh(g) = 2*sig(2g) - 1  (g rows pre-scaled by 2)
                    tg = stepp.tile([128, KH, GB], F32, tag=f"tg{g}",
                                    name=f"tg{g}")
                    nc.vector.tensor_scalar(tgs.append(tg) or tg[:],
                                            gacts[g][:, 4:6, :], 2.0, -1.0,
                                            ALU.mult, ALU.add)
                for g in range(NG):
                    ig = stepp.tile([128, KH, GB], F32, tag=f"ig{g}",
                                    name=f"ig{g}")
                    nc.vector.tensor_mul(igs.append(ig) or ig[:],
                                         gacts[g][:, 0:2, :], tgs[g][:])
                for g in range(NG):
                    fc_ = stepp.tile([128, KH, GB], F32, tag=f"fc{g}",
                                     name=f"fc{g}")
                    nc.vector.tensor_mul(
                        fcs.append(fc_) or fc_[:], gacts[g][:, 2:4, :],
                        c_state[:, :, g * GB:(g + 1) * GB])
                for g in range(NG):
                    nc.vector.tensor_add(c_state[:, :, g * GB:(g + 1) * GB],
                                         fcs[g][:], igs[g][:])
                for g in range(NG):
                    tc_t = stepp.tile([128, KH, GB], F32, tag=f"tc{g}",
                                      name=f"tc{g}")
                    nc.scalar.activation(tcs.append(tc_t) or tc_t[:],
                                         c_state[:, :, g * GB:(g + 1) * GB],
                                         AF.Tanh)
                hds = []
                for g in range(NG):
                    bs = slice(g * GB, (g + 1) * GB)
                    if layer == 0:
                        hdst = h0_sb[:, :, tau + 1, bs]
                    else:
                        hdst = h1r[:, tau % 2, :, bs]
                    nc.vector.tensor_mul(hdst, gacts[g][:, 6:8, :], tcs[g][:])
                    hds.append(hdst)
                if layer == 1:
                    for g in range(NG):
                        bs = slice(g * GB, (g + 1) * GB)
                        nc.vector.tensor_add(h1sum[:, :, bs],
                                             h1sum[:, :, bs], hds[g])

            # ---------- layer 0 ----------
            nc.gpsimd.memset(c_state[:], 0.0)
            for c in range(nch):
                xp_t = xp_gemm(0, c)
                for tl in range(CHUNK):
                    lstm_step(0, c * CHUNK + tl, xp_t, whh0_sb)
                # stage + exchange this chunk
                nc.sync.dma_start(
                    excin[c][:],
                    h0_sb[:, :, 1 + c * CHUNK:1 + (c + 1) * CHUNK, :])
                if use_collective:
                    nc.gpsimd.collective_compute(
                        "AllGather", ALU.bypass,
                        replica_groups=groups,
                        ins=[excin[c][:]], outs=[excout[c][:]])
                else:
                    nc.sync.dma_start(excout[c][0], excin[c][:])
                    nc.sync.dma_start(excout[c][1], excin[c][:])

            # ---------- layer 1 ----------
            nc.gpsimd.memset(c_state[:], 0.0)
            for c in range(nch):
                xp_t = xp_gemm(1, c)
                for tl in range(CHUNK):
                    lstm_step(1, c * CHUNK + tl, xp_t, whh1_sb)

            # ---------- pool + FC ----------
            pooled = stepp.tile([128, KH, BSH], BF16, tag="pooled")
            nc.scalar.activation(pooled[:], h1sum[:], AF.Identity,
                                 scale=1.0 / t_len)
            fcps = xpsp.tile([BSH, NCLS], F32, tag="fcps")
            for k in range(KH):
                nc.tensor.matmul(fcps[:], pooled[:, k, :], fcw_sb[:, k, :],
                                 start=(k == 0), stop=(k == KH - 1))
            out_sb = stepp.tile([BSH, NCLS], F32, tag="outsb")
            nc.vector.tensor_add(out_sb[:], fcps[:], fcb_sb[:])
            nc.sync.dma_start(out_ext[:], out_sb[:])

    _split_multi_waits(nc)
    return nc


def make_in_maps(x, w_ih0, w_hh0, b_ih0, b_hh0, w_ih1, w_hh1, b_ih1, b_hh1,
                 fc_w, fc_b, t_len=T):
    f32 = np.float32
    gsl = slice(2 * H, 3 * H)       # g-gate rows along the 4H axis
    in_maps = []
    for r in range(NCORES):
        q, d = r // 2, r % 2
        xs = np.asarray(x[BSH * q:BSH * q + BSH, :t_len], dtype=f32)
        if d == 1:
            xs = xs[:, ::-1, :]
        xT = np.concatenate([
            xs.transpose(2, 1, 0).reshape(DIN, t_len * BSH),
            np.ones((1, t_len * BSH), dtype=f32)], axis=0)

        whh0T = np.asarray(w_hh0[d], dtype=f32).T.copy()   # [H, 4H]
        whh0T[:, gsl] *= 2.0
        whh1T = np.asarray(w_hh1[d], dtype=f32).T.copy()
        whh1T[:, gsl] *= 2.0

        wih0T = np.concatenate([
            np.asarray(w_ih0[d], dtype=f32).T,
            (np.asarray(b_ih0[d]) + np.asarray(b_hh0[d]))
            .astype(f32)[None, :]], axis=0)                # [13, 4H]
        wih0T[:, gsl] *= 2.0

        wih1e = np.zeros((3 * H, H4), dtype=f32)
        w1 = np.asarray(w_ih1[d], dtype=f32)               # [4H, 2H]
        wih1e[0:H] = w1[:, d * H:(d + 1) * H].T
        if d == 1:
            wih1e[H:2 * H] = w1[:, 0:H].T                  # fwd-slot features
        else:
            wih1e[2 * H:3 * H] = w1[:, H:2 * H].T          # bwd-slot features
        wih1e[:, gsl] *= 2.0

        b1row = (np.asarray(b_ih1[d]) + np.asarray(b_hh1[d])).astype(f32)
        b1row = b1row[None, :].copy()
        b1row[:, gsl] *= 2.0

        fcb_t = (np.tile(np.asarray(fc_b, dtype=f32), (BSH, 1))
                 if d == 0 else np.zeros((BSH, NCLS), dtype=f32))
        in_maps.append({
            "xT": np.ascontiguousarray(xT),
            "whh0T": np.ascontiguousarray(whh0T),
            "wih0T": np.ascontiguousarray(wih0T),
            "whh1T": np.ascontiguousarray(whh1T),
            "wih1Te": wih1e,
            "b1": b1row,
            "ident": np.eye(128, dtype=f32),
            "fcwT": np.ascontiguousarray(
                np.asarray(fc_w, dtype=f32)[:, d * H:(d + 1) * H].T),
            "fcb": fcb_t,
            "out": np.zeros((BSH, NCLS), dtype=f32),
        })
    return in_maps


_NC_CACHE = {}


def kernel(x, w_ih0, w_hh0, b_ih0, b_hh0, w_ih1, w_hh1, b_ih1, b_hh1,
           fc_w, fc_b, trace=False):
    if T not in _NC_CACHE:
        _NC_CACHE[T] = build_nc(T)
    nc = _NC_CACHE[T]
    in_maps = make_in_maps(x, w_ih0, w_hh0, b_ih0, b_hh0, w_ih1, w_hh1,
                           b_ih1, b_hh1, fc_w, fc_b)
    res = run_bass_kernel_spmd(nc, in_maps, list(range(NCORES)), trace=trace)
    out = np.zeros((B, NCLS), dtype=np.float32)
    for q in range(NCORES // 2):
        out[BSH * q:BSH * q + BSH] = (res.results[2 * q]["out"]
                                      + res.results[2 * q + 1]["out"])
    kernel.last_result = res
    return out

